# revision 1
# baseline (speedup 1.0000x reference)
"""Trainium2 Bass kernel for nn_EquiStructureDecoder (8-core SPMD).

Key algebraic fact used: the network's outputs (coord_pred, attr_pred,
global_pred) depend only on the hidden stream h.  In each block,
h <- h + softmax(qk^T/sqrt(D)) @ v  uses only h; the coordinate stream x
(rel_x / edge_feat / coord_w / delta_x) never feeds back into h and is
not part of the returned outputs, so it is dead code and is not computed.
This is exact (bitwise dataflow equivalence of the h path), not an
approximation.

Distribution (row-parallel over queries, per the sharding hint):
  - each of the 8 cores owns a 128-row slab of h
  - k/v are computed from the full (replicated) h each layer
  - after each block the updated slabs are AllGather'd (bf16 for the
    first two layers; fp32 after the last layer, feeding the
    segment-mean global head)
"""

import sys
import os

for _p in ("/opt/trn_rl_repo",):
    if _p not in sys.path:
        sys.path.insert(0, _p)

import numpy as np
import ml_dtypes

import concourse.bass as bass
import concourse.bacc as bacc
import concourse.tile as tile
from concourse import mybir
from concourse import bass_utils

N = 1024
D = 128
NC = 8
S = N // NC        # 128 rows per core
L = 3
NG = 8
A = 16
G = 8
INV_SQRT_D = float(1.0 / np.sqrt(np.float32(D)))

F32 = mybir.dt.float32
BF16 = mybir.dt.bfloat16
AF = mybir.ActivationFunctionType
ALU = mybir.AluOpType

_BF = ml_dtypes.bfloat16


def _ts(i, size=128):
    return slice(i * size, (i + 1) * size)


def _build_program():
    nc = bacc.Bacc(
        "TRN2",
        target_bir_lowering=False,
        debug=False,
        enable_asserts=False,
        num_devices=NC,
    )

    def din(name, shape, dt=F32):
        return nc.dram_tensor(name, list(shape), dt, kind="ExternalInput").ap()

    def dout(name, shape, dt=F32):
        return nc.dram_tensor(name, list(shape), dt, kind="ExternalOutput").ap()

    # ---- replicated inputs ----
    cw_f = din("cw_f", (2, D))            # coord_enc_W
    aw_f = din("aw_f", (A, D))            # attr_enc_W
    tw_f = din("tw_f", (1, D))            # time_W
    cw_b = din("cw_b", (2, D), BF16)
    aw_b = din("aw_b", (A, D), BF16)
    tw_b = din("tw_b", (1, D), BF16)
    coordT_b = din("coordT_b", (2, N), BF16)
    attrT_b = din("attrT_b", (A, N), BF16)
    tT_b = din("tT_b", (1, N), BF16)
    condTpb_f = din("condTpb_f", (D, N))          # condition.T + enc biases
    wq_b = din("wq_b", (L, D, D), BF16)
    wk_b = din("wk_b", (L, D, D), BF16)
    wv_b = din("wv_b", (L, D, D), BF16)
    bq_f = din("bq_f", (L, D))
    bk_f = din("bk_f", (L, D))
    bv_f = din("bv_f", (L, D))
    i128_f = din("i128_f", (D, D))                # identity
    cow_f = din("cow_f", (D, 2))                  # coord_out_W
    cob_f = din("cob_f", (S, 2))                  # coord_out_b broadcast
    aow_f = din("aow_f", (D, A))
    aob_f = din("aob_f", (S, A))
    gw1_f = din("gw1_f", (D, D))
    gw2_f = din("gw2_f", (D, G))
    gb1_f = din("gb1_f", (NG, D))
    gb2_f = din("gb2_f", (NG, G))
    meanMT_f = din("meanMT_f", (NC, S, NG))       # block b: meanM[g, b*128+i].T

    # ---- per-core inputs ----
    coordT_own = din("coordT_own", (2, S))
    attrT_own = din("attrT_own", (A, S))
    tT_own = din("tT_own", (1, S))
    condTpb_own = din("condTpb_own", (D, S))

    # ---- outputs (own slab; global replicated) ----
    o_coord = dout("o_coord", (S, 2))
    o_attr = dout("o_attr", (S, A))
    o_global = dout("o_global", (NG, G))

    with tile.TileContext(nc) as tc:
        with (
            tc.tile_pool(name="const", bufs=1) as cp,
            tc.tile_pool(name="work", bufs=2) as wp,
            tc.tile_pool(name="psum", bufs=1, space="PSUM") as pp,
            tc.tile_pool(name="dram", bufs=1, space="DRAM") as dp,
        ):
            # ---------- load constants into SBUF ----------
            def load(name, ap, shape, dt):
                t = cp.tile(list(shape), dt, name=name)
                nc.sync.dma_start(t[:], ap)
                return t

            cw_fs = load("cw_fs", cw_f, (2, D), F32)
            aw_fs = load("aw_fs", aw_f, (A, D), F32)
            tw_fs = load("tw_fs", tw_f, (1, D), F32)
            cw_bs = load("cw_bs", cw_b, (2, D), BF16)
            aw_bs = load("aw_bs", aw_b, (A, D), BF16)
            tw_bs = load("tw_bs", tw_b, (1, D), BF16)
            coordT_bs = load("coordT_bs", coordT_b, (2, N), BF16)
            attrT_bs = load("attrT_bs", attrT_b, (A, N), BF16)
            tT_bs = load("tT_bs", tT_b, (1, N), BF16)
            condTpb_fs = load("condTpb_fs", condTpb_f, (D, N), F32)
            wq_s = load("wq_s", wq_b.rearrange("l k d -> k l d"), (D, L, D), BF16)
            wk_s = load("wk_s", wk_b.rearrange("l k d -> k l d"), (D, L, D), BF16)
            wv_s = load("wv_s", wv_b.rearrange("l k d -> k l d"), (D, L, D), BF16)
            bq_s = load("bq_s", bq_f.rearrange("l d -> d l"), (D, L), F32)
            bk_s = load("bk_s", bk_f.rearrange("l d -> d l"), (D, L), F32)
            bv_s = load("bv_s", bv_f.rearrange("l d -> d l"), (D, L), F32)
            i128_s = load("i128_s", i128_f, (D, D), F32)
            cow_s = load("cow_s", cow_f, (D, 2), F32)
            cob_s = load("cob_s", cob_f, (S, 2), F32)
            aow_s = load("aow_s", aow_f, (D, A), F32)
            aob_s = load("aob_s", aob_f, (S, A), F32)
            gw1_s = load("gw1_s", gw1_f, (D, D), F32)
            gw2_s = load("gw2_s", gw2_f, (D, G), F32)
            gb1_s = load("gb1_s", gb1_f, (NG, D), F32)
            gb2_s = load("gb2_s", gb2_f, (NG, G), F32)
            meanMT_s = load(
                "meanMT_s", meanMT_f.rearrange("b i g -> i b g"), (S, NC, NG), F32
            )
            coordT_os = load("coordT_os", coordT_own, (2, S), F32)
            attrT_os = load("attrT_os", attrT_own, (A, S), F32)
            tT_os = load("tT_os", tT_own, (1, S), F32)
            condTpb_os = load("condTpb_os", condTpb_own, (D, S), F32)

            # ---------- h0 ----------
            # own slab, fp32 exact: h_ownT[d, i'] (residual carry)
            h_ownT = cp.tile([D, S], F32, name="h_ownT")
            ps0 = pp.tile([D, S], F32, tag="small", name="ps0")
            nc.tensor.matmul(ps0[:], cw_fs[:], coordT_os[:], start=True, stop=False)
            nc.tensor.matmul(ps0[:], aw_fs[:], attrT_os[:], start=False, stop=False)
            nc.tensor.matmul(ps0[:], tw_fs[:], tT_os[:], start=False, stop=True)
            nc.vector.tensor_tensor(h_ownT[:], ps0[:], condTpb_os[:], op=ALU.add)

            # full h0, bf16 (feeds k/v of layer 1 only)
            hT_full = cp.tile([D, N], BF16, name="hT_full")
            for c in range(2):
                psf = pp.tile([D, 512], F32, tag="big", name="psf")
                sl = slice(c * 512, (c + 1) * 512)
                nc.tensor.matmul(psf[:], cw_bs[:], coordT_bs[:, sl], start=True, stop=False)
                nc.tensor.matmul(psf[:], aw_bs[:], attrT_bs[:, sl], start=False, stop=False)
                nc.tensor.matmul(psf[:], tw_bs[:], tT_bs[:, sl], start=False, stop=True)
                nc.vector.tensor_tensor(
                    hT_full[:, sl], psf[:], condTpb_fs[:, sl], op=ALU.add
                )

            h_ownT_b = cp.tile([D, S], BF16, name="h_ownT_b")
            nc.scalar.activation(h_ownT_b[:], h_ownT[:], AF.Copy)

            # ---------- attention blocks ----------
            for l in range(L):
                last = l == L - 1

                # q (own slab):  qT[d,i'] = Wq^T @ h_ownT   (+bq per-partition)
                ps_q = pp.tile([D, S], F32, tag="small", name="ps_q")
                nc.tensor.matmul(ps_q[:], wq_s[:, l], h_ownT_b[:], start=True, stop=True)
                qT_b = wp.tile([D, S], BF16, name="qT_b")
                nc.scalar.activation(qT_b[:], ps_q[:], AF.Identity, bias=bq_s[:, l : l + 1])

                # k (full):  kT[d,j] = Wk^T @ hT_full  (+bk)
                ps_k = pp.tile([D, N], F32, tag="big", name="ps_k")
                for c in range(2):
                    sl = slice(c * 512, (c + 1) * 512)
                    nc.tensor.matmul(
                        ps_k[:, sl], wk_s[:, l], hT_full[:, sl], start=True, stop=True
                    )
                kT_b = wp.tile([D, N], BF16, name="kT_b")
                nc.vector.tensor_scalar_add(kT_b[:], ps_k[:], bk_s[:, l : l + 1])

                # v (full, untransposed):  v[j,d] = h_j @ Wv   (bv folded in later)
                ps_v = pp.tile([D, N], F32, tag="big", name="ps_v")
                for b in range(NC):
                    nc.tensor.matmul(
                        ps_v[:, _ts(b)], hT_full[:, _ts(b)], wv_s[:, l],
                        start=True, stop=True,
                    )
                v_b = wp.tile([D, N], BF16, name="v_b")
                nc.scalar.activation(v_b[:], ps_v[:], AF.Copy)

                # scores S[i',j] = qT^T @ kT   (scale folded into exp)
                ps_s = pp.tile([S, N], F32, tag="big", name="ps_s")
                for c in range(2):
                    sl = slice(c * 512, (c + 1) * 512)
                    nc.tensor.matmul(
                        ps_s[:, sl], qT_b[:], kT_b[:, sl], start=True, stop=True
                    )

                # softmax (scores are tiny for this model; no max-subtraction
                # needed): E = exp(S/sqrt(D)), rowsum via accum_out
                e_f = wp.tile([S, N], F32, name="e_f")
                rowsum = wp.tile([S, 1], F32, name="rowsum")
                nc.scalar.activation(
                    e_f[:], ps_s[:], AF.Exp, scale=INV_SQRT_D, accum_out=rowsum[:]
                )
                recip = wp.tile([S, 1], F32, name="recip")
                nc.vector.reciprocal(recip[:], rowsum[:])

                # normalize rows, then transpose E -> ET blocks
                e_n = wp.tile([S, N], F32, name="e_n")
                nc.vector.tensor_scalar_mul(e_n[:], e_f[:], recip[:])
                ps_et = pp.tile([S, N], F32, tag="big", name="ps_et")
                for b in range(NC):
                    nc.tensor.transpose(ps_et[:, _ts(b)], e_n[:, _ts(b)], i128_s[:])
                et_b = wp.tile([S, N], BF16, name="et_b")
                nc.vector.tensor_copy(et_b[:], ps_et[:])

                # aggT[d,i'] = sum_b v_b^T @ ET_b  == (attn @ v)^T
                ps_a = pp.tile([D, S], F32, tag="small", name="ps_a")
                for b in range(NC):
                    nc.tensor.matmul(
                        ps_a[:], v_b[:, _ts(b)], et_b[:, _ts(b)],
                        start=(b == 0), stop=(b == NC - 1),
                    )

                # h_ownT <- h_ownT + aggT + bv   (softmax rows sum to 1)
                h_new = cp.tile([D, S], F32, name=f"h_new{l}")
                nc.vector.scalar_tensor_tensor(
                    h_new[:], ps_a[:], bv_s[:, l : l + 1], h_ownT[:],
                    op0=ALU.add, op1=ALU.add,
                )
                h_ownT = h_new
                if not last:
                    h_ownT_b = wp.tile([D, S], BF16, name="h_ownT_b2")
                    nc.scalar.activation(h_ownT_b[:], h_ownT[:], AF.Copy)

                # stage own slab (untransposed) for the AllGather
                ps_h = pp.tile([D, S], F32, tag="small", name="ps_h")
                nc.tensor.transpose(ps_h[:], h_ownT[:], i128_s[:])
                ag_dt = F32 if last else BF16
                stage = wp.tile([S, D], ag_dt, name="stage")
                nc.vector.tensor_copy(stage[:], ps_h[:])

                ag_in = dp.tile([S, D], ag_dt, name=f"ag_in{l}")
                ag_out = dp.tile([N, D], ag_dt, name=f"ag_out{l}", addr_space="Shared")
                nc.sync.dma_start(ag_in[:], stage[:])
                nc.gpsimd.collective_compute(
                    "AllGather",
                    ALU.bypass,
                    replica_groups=[list(range(NC))],
                    ins=[ag_in[:]],
                    outs=[ag_out[:]],
                )
                if not last:
                    hT_full = wp.tile([D, N], BF16, name="hT_full2")
                    nc.sync.dma_start_transpose(hT_full[:], ag_out[:])
                else:
                    ag_final = ag_out

            # ---------- output heads (own slab, fp32) ----------
            ps_c = pp.tile([S, 2], F32, tag="small", name="ps_c")
            nc.tensor.matmul(ps_c[:], h_ownT[:], cow_s[:], start=True, stop=True)
            oc_s = wp.tile([S, 2], F32, name="oc_s")
            nc.vector.tensor_tensor(oc_s[:], ps_c[:], cob_s[:], op=ALU.add)
            nc.sync.dma_start(o_coord, oc_s[:])

            ps_at = pp.tile([S, A], F32, tag="small", name="ps_at")
            nc.tensor.matmul(ps_at[:], h_ownT[:], aow_s[:], start=True, stop=True)
            oa_s = wp.tile([S, A], F32, name="oa_s")
            nc.vector.tensor_tensor(oa_s[:], ps_at[:], aob_s[:], op=ALU.add)
            nc.sync.dma_start(o_attr, oa_s[:])

            # ---------- global head ----------
            # h_graphT[d,g] = sum_b h_b^T @ meanMT_b
            ps_g = pp.tile([D, NG], F32, tag="small", name="ps_g")
            for b in range(NC):
                h_blk = wp.tile([S, D], F32, name="h_blk")
                nc.sync.dma_start(h_blk[:], ag_final[_ts(b), :])
                nc.tensor.matmul(
                    ps_g[:], h_blk[:], meanMT_s[:, b],
                    start=(b == 0), stop=(b == NC - 1),
                )
            hgT = wp.tile([D, NG], F32, name="hgT")
            nc.scalar.activation(hgT[:], ps_g[:], AF.Copy)

            # z1 = relu(h_graph @ gW1 + gb1)
            ps_z1 = pp.tile([NG, D], F32, tag="small", name="ps_z1")
            nc.tensor.matmul(ps_z1[:], hgT[:], gw1_s[:], start=True, stop=True)
            z1 = wp.tile([NG, D], F32, name="z1")
            nc.vector.tensor_tensor(z1[:], ps_z1[:], gb1_s[:], op=ALU.add)
            z1r = wp.tile([NG, D], F32, name="z1r")
            nc.vector.tensor_scalar_max(z1r[:], z1[:], 0.0)

            ps_z1t = pp.tile([D, NG], F32, tag="small", name="ps_z1t")
            nc.tensor.transpose(ps_z1t[:], z1r[:], i128_s[:NG, :NG])
            z1t = wp.tile([D, NG], F32, name="z1t")
            nc.scalar.activation(z1t[:], ps_z1t[:], AF.Copy)

            ps_z2 = pp.tile([NG, G], F32, tag="small", name="ps_z2")
            nc.tensor.matmul(ps_z2[:], z1t[:], gw2_s[:], start=True, stop=True)
            og_s = wp.tile([NG, G], F32, name="og_s")
            nc.vector.tensor_tensor(og_s[:], ps_z2[:], gb2_s[:], op=ALU.add)
            nc.sync.dma_start(o_global, og_s[:])

    nc.compile()
    return nc


_PROGRAM = None


def _get_program():
    global _PROGRAM
    if _PROGRAM is None:
        _PROGRAM = _build_program()
    return _PROGRAM


def _build_in_maps(inputs):
    f32 = np.float32
    coord = np.asarray(inputs["theta_coord"], f32)        # [N, 2]
    attr = np.asarray(inputs["theta_attr"], f32)          # [N, A]
    t = np.asarray(inputs["t"], f32)                      # [N, 1]
    cond = np.asarray(inputs["condition"], f32)           # [N, D]
    batch = np.asarray(inputs["batch"]).astype(np.int64)  # [N]

    cw = np.asarray(inputs["coord_enc_W"], f32)
    aw = np.asarray(inputs["attr_enc_W"], f32)
    tw = np.asarray(inputs["time_W"], f32)
    enc_b = (
        np.asarray(inputs["coord_enc_b"], f32)
        + np.asarray(inputs["attr_enc_b"], f32)
        + np.asarray(inputs["time_b"], f32)
    )
    condTpb = cond.T + enc_b[:, None]                     # [D, N]

    wq = np.asarray(inputs["Wq"], f32)
    wk = np.asarray(inputs["Wk"], f32)
    wv = np.asarray(inputs["Wv"], f32)
    bq = np.asarray(inputs["bq"], f32)
    bk = np.asarray(inputs["bk"], f32)
    bv = np.asarray(inputs["bv"], f32)

    counts = np.bincount(batch, minlength=NG).astype(f32)
    onehot = (batch[None, :] == np.arange(NG)[:, None]).astype(f32)  # [G, N]
    meanM = onehot / np.maximum(counts, 1.0)[:, None]
    meanMT = np.ascontiguousarray(
        meanM.T.reshape(NC, S, NG)                         # [b, i', g]
    )

    rep = {
        "cw_f": cw,
        "aw_f": aw,
        "tw_f": tw,
        "cw_b": cw.astype(_BF),
        "aw_b": aw.astype(_BF),
        "tw_b": tw.astype(_BF),
        "coordT_b": np.ascontiguousarray(coord.T).astype(_BF),
        "attrT_b": np.ascontiguousarray(attr.T).astype(_BF),
        "tT_b": np.ascontiguousarray(t.T).astype(_BF),
        "condTpb_f": np.ascontiguousarray(condTpb),
        "wq_b": wq.astype(_BF),
        "wk_b": wk.astype(_BF),
        "wv_b": wv.astype(_BF),
        "bq_f": bq,
        "bk_f": bk,
        "bv_f": bv,
        "i128_f": np.eye(D, dtype=f32),
        "cow_f": np.asarray(inputs["coord_out_W"], f32),
        "cob_f": np.broadcast_to(
            np.asarray(inputs["coord_out_b"], f32), (S, 2)
        ).copy(),
        "aow_f": np.asarray(inputs["attr_out_W"], f32),
        "aob_f": np.broadcast_to(
            np.asarray(inputs["attr_out_b"], f32), (S, A)
        ).copy(),
        "gw1_f": np.asarray(inputs["gW1"], f32),
        "gw2_f": np.asarray(inputs["gW2"], f32),
        "gb1_f": np.broadcast_to(np.asarray(inputs["gb1"], f32), (NG, D)).copy(),
        "gb2_f": np.broadcast_to(np.asarray(inputs["gb2"], f32), (NG, G)).copy(),
        "meanMT_f": meanMT,
    }

    in_maps = []
    for c in range(NC):
        sl = slice(c * S, (c + 1) * S)
        m = dict(rep)
        m["coordT_own"] = np.ascontiguousarray(coord[sl].T)
        m["attrT_own"] = np.ascontiguousarray(attr[sl].T)
        m["tT_own"] = np.ascontiguousarray(t[sl].T)
        m["condTpb_own"] = np.ascontiguousarray(condTpb[:, sl])
        in_maps.append(m)
    return in_maps


def kernel(**inputs):
    nc = _get_program()
    in_maps = _build_in_maps(inputs)
    res = bass_utils.run_bass_kernel_spmd(nc, in_maps, core_ids=list(range(NC)))
    coord_pred = np.concatenate(
        [res.results[c]["o_coord"] for c in range(NC)], axis=0
    )
    attr_pred = np.concatenate(
        [res.results[c]["o_attr"] for c in range(NC)], axis=0
    )
    global_pred = res.results[0]["o_global"]
    return (
        coord_pred.astype(np.float32),
        attr_pred.astype(np.float32),
        global_pred.astype(np.float32),
    )


# revision 3
# speedup vs baseline: 1.1085x; 1.1085x over previous
"""Trainium2 Bass kernel for nn_EquiStructureDecoder (8-core SPMD).

Key algebraic fact used: the network's outputs (coord_pred, attr_pred,
global_pred) depend only on the hidden stream h.  In each block,
h <- h + softmax(qk^T/sqrt(D)) @ v  uses only h; the coordinate stream x
(rel_x / edge_feat / coord_w / delta_x) never feeds back into h and is
not part of the returned outputs, so it is dead code and is not computed.
This is exact (dataflow equivalence of the h path), not an approximation.

Distribution (row-parallel over queries, per the sharding hint):
  - each of the 8 cores owns a 128-row slab of h (carried transposed,
    hT[d, i'], fp32 residual; bf16 operands for matmuls)
  - k/v are computed from the full (replicated) h each layer
  - after blocks 1 and 2 the updated transposed slabs are AllGather'd
    (bf16); the gather output is directly the stacked hT blocks
  - the global head needs only segment-sums of h, which are linear:
    each core computes its partial [D, G] and one 4KB AllReduce(add)
    replaces a full AllGather
  - softmax is computed without max-subtraction (scores for this model
    are O(0.1); exp is safe in fp32) and rows are normalized at the
    residual update via an outer-product broadcast of 1/rowsum
"""

import sys

for _p in ("/opt/trn_rl_repo",):
    if _p not in sys.path:
        sys.path.insert(0, _p)

import numpy as np
import ml_dtypes

import concourse.bass as bass
import concourse.bacc as bacc
import concourse.tile as tile
from concourse import mybir
from concourse import bass_utils

N = 1024
D = 128
NC = 8
S = N // NC        # 128 rows per core
L = 3
NG = 8
A = 16
G = 8
INV_SQRT_D = float(1.0 / np.sqrt(np.float32(D)))

F32 = mybir.dt.float32
BF16 = mybir.dt.bfloat16
AF = mybir.ActivationFunctionType
ALU = mybir.AluOpType

_BF = ml_dtypes.bfloat16


def _ts(i, size=128):
    return slice(i * size, (i + 1) * size)


class _Blob:
    """Column-packed constant blob: host array + SBUF slice bookkeeping."""

    def __init__(self, parts, dtype):
        self.parts = parts
        self.dtype = dtype
        self.cols = 0
        self.sections = {}
        self.arrays = []

    def add(self, name, arr):
        arr = np.asarray(arr)
        rows, cols = arr.shape
        assert rows <= self.parts
        self.sections[name] = (self.cols, cols, rows)
        self.cols += cols
        self.arrays.append(arr)
        return name

    def build(self):
        out = np.zeros((self.parts, self.cols), dtype=self.dtype)
        for (name, (off, cols, rows)), arr in zip(
            self.sections.items(), self.arrays
        ):
            out[:rows, off : off + cols] = arr
        return out


# blob section layouts (host + device must agree); filled in _build_in_maps
_B32 = None   # [128, *] f32, per-core
_BBF = None   # [128, *] bf16, replicated
_S32 = None   # [16, *] f32, per-core
_SBF = None   # [16, *] bf16, replicated


def _make_blob_layouts():
    """Define blob column layouts with dummy arrays (shapes only)."""
    b32 = _Blob(128, np.float32)
    b32.add("condTpb_own", np.zeros((D, S)))
    b32.add("i128", np.zeros((D, D)))
    b32.add("cow", np.zeros((D, 2)))
    b32.add("cob", np.zeros((S, 2)))
    b32.add("aow", np.zeros((D, A)))
    b32.add("aob", np.zeros((S, A)))
    b32.add("gw1", np.zeros((D, D)))
    b32.add("gw2", np.zeros((D, G)))
    b32.add("gb1", np.zeros((NG, D)))
    b32.add("gb2", np.zeros((NG, G)))
    b32.add("bq", np.zeros((D, L)))
    b32.add("bk", np.zeros((D, L)))
    b32.add("bv", np.zeros((D, L)))
    b32.add("meanMT_own", np.zeros((S, NG)))

    bbf = _Blob(128, _BF)
    bbf.add("wq", np.zeros((D, L * D)))
    bbf.add("wk", np.zeros((D, L * D)))
    bbf.add("wv", np.zeros((D, L * D)))
    bbf.add("condTpb", np.zeros((D, N)))
    bbf.add("i128b", np.zeros((D, D)))

    s32 = _Blob(16, np.float32)
    s32.add("coordT_own", np.zeros((2, S)))
    s32.add("attrT_own", np.zeros((A, S)))
    s32.add("tT_own", np.zeros((1, S)))
    s32.add("cw", np.zeros((2, D)))
    s32.add("aw", np.zeros((A, D)))
    s32.add("tw", np.zeros((1, D)))
    s32.add("ones", np.zeros((1, D)))

    sbf = _Blob(16, _BF)
    sbf.add("coordT", np.zeros((2, N)))
    sbf.add("attrT", np.zeros((A, N)))
    sbf.add("tT", np.zeros((1, N)))
    sbf.add("cwb", np.zeros((2, D)))
    sbf.add("awb", np.zeros((A, D)))
    sbf.add("twb", np.zeros((1, D)))
    return b32, bbf, s32, sbf


_LAYOUT = _make_blob_layouts()


def _build_program():
    b32l, bbfl, s32l, sbfl = _LAYOUT
    nc = bacc.Bacc(
        "TRN2",
        target_bir_lowering=False,
        debug=False,
        enable_asserts=False,
        num_devices=NC,
    )

    blob32 = nc.dram_tensor("blob32", [128, b32l.cols], F32, kind="ExternalInput").ap()
    blobbf = nc.dram_tensor("blobbf", [128, bbfl.cols], BF16, kind="ExternalInput").ap()
    sm32 = nc.dram_tensor("sm32", [16, s32l.cols], F32, kind="ExternalInput").ap()
    smbf = nc.dram_tensor("smbf", [16, sbfl.cols], BF16, kind="ExternalInput").ap()

    o_coord = nc.dram_tensor("o_coord", [S, 2], F32, kind="ExternalOutput").ap()
    o_attr = nc.dram_tensor("o_attr", [S, A], F32, kind="ExternalOutput").ap()
    o_global = nc.dram_tensor("o_global", [NG, G], F32, kind="ExternalOutput").ap()

    with tile.TileContext(nc) as tc:
        with (
            tc.tile_pool(name="const", bufs=1) as cp,
            tc.tile_pool(name="work", bufs=2) as wp,
            tc.tile_pool(name="psum", bufs=1, space="PSUM") as pp,
            tc.tile_pool(name="dram", bufs=1, space="DRAM") as dp,
        ):
            # ---------- constant blobs: 4 DMAs on 2 HWDGE queues ----------
            t32 = cp.tile([128, b32l.cols], F32, name="t32")
            tbf = cp.tile([128, bbfl.cols], BF16, name="tbf")
            u32 = cp.tile([16, s32l.cols], F32, name="u32")
            ubf = cp.tile([16, sbfl.cols], BF16, name="ubf")
            nc.scalar.dma_start(u32[:], sm32)
            nc.sync.dma_start(ubf[:], smbf)
            nc.scalar.dma_start(t32[:], blob32)
            nc.sync.dma_start(tbf[:], blobbf)

            def c32(name):
                off, cols, rows = b32l.sections[name]
                return t32[:rows, off : off + cols]

            def cbf(name):
                off, cols, rows = bbfl.sections[name]
                return tbf[:rows, off : off + cols]

            def c16(name):
                off, cols, rows = s32l.sections[name]
                return u32[:rows, off : off + cols]

            def c16b(name):
                off, cols, rows = sbfl.sections[name]
                return ubf[:rows, off : off + cols]

            def wsl(name, l):
                off, _, _ = bbfl.sections[name]
                return tbf[:, off + l * D : off + (l + 1) * D]

            def bsl(name, l):
                off, _, _ = b32l.sections[name]
                return t32[:, off + l : off + l + 1]

            # ---------- h0 ----------
            # full h0 (bf16) straight into the hT layout used by k/v
            hT_all = cp.tile([D, N], BF16, name="hT_all0")
            for c in range(2):
                psf = pp.tile([D, 512], F32, tag="big", name="psf")
                sl = slice(c * 512, (c + 1) * 512)
                nc.tensor.matmul(psf[:], c16b("cwb"), c16b("coordT")[:, sl], start=True, stop=False)
                nc.tensor.matmul(psf[:], c16b("awb"), c16b("attrT")[:, sl], start=False, stop=False)
                nc.tensor.matmul(psf[:], c16b("twb"), c16b("tT")[:, sl], start=False, stop=True)
                nc.vector.tensor_tensor(hT_all[:, sl], psf[:], cbf("condTpb")[:, sl], op=ALU.add)

            # own slab fp32 (exact residual carry)
            h_ownT = cp.tile([D, S], F32, name="h_ownT")
            ps0 = pp.tile([D, S], F32, tag="small", name="ps0")
            nc.tensor.matmul(ps0[:], c16("cw"), c16("coordT_own"), start=True, stop=False)
            nc.tensor.matmul(ps0[:], c16("aw"), c16("attrT_own"), start=False, stop=False)
            nc.tensor.matmul(ps0[:], c16("tw"), c16("tT_own"), start=False, stop=True)
            nc.vector.tensor_tensor(h_ownT[:], ps0[:], c32("condTpb_own"), op=ALU.add)
            h_ownT_b = cp.tile([D, S], BF16, name="h_ownT_b0")
            nc.scalar.activation(h_ownT_b[:], h_ownT[:], AF.Copy)

            # ---------- attention blocks ----------
            for l in range(L):
                last = l == L - 1

                if l > 0:
                    # rebuild hT_all from the AllGather output (stacked
                    # transposed blocks); split across both HWDGE queues
                    hT_all = wp.tile([D, N], BF16, name="hT_all")
                    for b in range(NC):
                        eng = nc.sync if b % 2 == 0 else nc.scalar
                        eng.dma_start(hT_all[:, _ts(b)], ag_out[_ts(b), :])

                # q (own): qT[d,i'] = Wq^T @ h_ownT (+bq); ready during AG
                ps_q = pp.tile([D, S], F32, tag="small", name="ps_q")
                nc.tensor.matmul(ps_q[:], wsl("wq", l), h_ownT_b[:], start=True, stop=True)
                qT_b = wp.tile([D, S], BF16, name="qT_b")
                nc.scalar.activation(qT_b[:], ps_q[:], AF.Identity, bias=bsl("bq", l))

                # k: kT[d,j] = Wk^T @ hT (+bk), 2 chunks pipelined into S
                ps_k = pp.tile([D, N], F32, tag="big", name="ps_k")
                kT_b = wp.tile([D, N], BF16, name="kT_b")
                ps_s = pp.tile([S, N], F32, tag="big", name="ps_s")
                for c in range(2):
                    sl = slice(c * 512, (c + 1) * 512)
                    nc.tensor.matmul(ps_k[:, sl], wsl("wk", l), hT_all[:, sl], start=True, stop=True)
                    nc.vector.tensor_scalar_add(kT_b[:, sl], ps_k[:, sl], bsl("bk", l))
                    nc.tensor.matmul(ps_s[:, sl], qT_b[:], kT_b[:, sl], start=True, stop=True)

                # v[j,d] (untransposed; bv folded into the update)
                ps_v = pp.tile([D, N], F32, tag="big", name="ps_v")
                for b in range(NC):
                    nc.tensor.matmul(ps_v[:, _ts(b)], hT_all[:, _ts(b)], wsl("wv", l), start=True, stop=True)
                v_b = wp.tile([D, N], BF16, name="v_b")
                nc.scalar.activation(v_b[:], ps_v[:], AF.Copy)

                # E = exp(S/sqrt(D)) unnormalized (bf16) + fp32 rowsums
                e_b = wp.tile([S, N], BF16, name="e_b")
                rowsum = wp.tile([S, 1], F32, name="rowsum")
                nc.scalar.activation(e_b[:], ps_s[:], AF.Exp, scale=INV_SQRT_D, accum_out=rowsum[:])

                # recip broadcast tile rb[d, i'] = 1/rowsum[i']  (off-chain)
                recip = wp.tile([S, 1], F32, name="recip")
                nc.vector.reciprocal(recip[:], rowsum[:])
                ps_rt = pp.tile([1, S], F32, tag="small", name="ps_rt")
                nc.tensor.transpose(ps_rt[:], recip[:], c32("i128"))
                rt_s = wp.tile([1, S], F32, name="rt_s")
                nc.vector.tensor_copy(rt_s[:], ps_rt[:])
                ps_rb = pp.tile([D, S], F32, tag="small", name="ps_rb")
                nc.tensor.matmul(ps_rb[:], c16("ones"), rt_s[:], start=True, stop=True)
                rb_s = wp.tile([D, S], F32, name="rb_s")
                nc.vector.tensor_copy(rb_s[:], ps_rb[:])

                # ET blocks (transpose unnormalized E)
                ps_et = pp.tile([S, N], BF16, tag="big", name="ps_et")
                for b in range(NC):
                    nc.tensor.transpose(ps_et[:, _ts(b)], e_b[:, _ts(b)], cbf("i128b"))
                et_b = wp.tile([S, N], BF16, name="et_b")
                for c in range(2):
                    sl = slice(c * 512, (c + 1) * 512)
                    nc.vector.tensor_copy(et_b[:, sl], ps_et[:, sl])

                # aggT[d,i'] = sum_b v_b^T @ ET_b  == (E @ v)^T
                ps_a = pp.tile([D, S], F32, tag="small", name="ps_a")
                for b in range(NC):
                    nc.tensor.matmul(
                        ps_a[:], v_b[:, _ts(b)], et_b[:, _ts(b)],
                        start=(b == 0), stop=(b == NC - 1),
                    )

                # h <- h + aggT*rb + bv
                t1 = wp.tile([D, S], F32, name="t1")
                nc.vector.tensor_tensor(t1[:], ps_a[:], rb_s[:], op=ALU.mult)
                h_new = cp.tile([D, S], F32, name=f"h_new{l}")
                nc.vector.scalar_tensor_tensor(
                    h_new[:], t1[:], bsl("bv", l), h_ownT[:], op0=ALU.add, op1=ALU.add
                )
                h_ownT = h_new
                h_ownT_b = wp.tile([D, S], BF16, name="h_ownT_b")
                nc.scalar.activation(h_ownT_b[:], h_ownT[:], AF.Copy)

                if not last:
                    ag_in = dp.tile([D, S], BF16, name=f"ag_in{l}")
                    ag_out = dp.tile([N, S], BF16, name=f"ag_out{l}", addr_space="Shared")
                    nc.sync.dma_start(ag_in[:], h_ownT_b[:])
                    nc.gpsimd.collective_compute(
                        "AllGather",
                        ALU.bypass,
                        replica_groups=[list(range(NC))],
                        ins=[ag_in[:]],
                        outs=[ag_out[:]],
                    )

            # ---------- global head partial + AllReduce (fires first) ----------
            ps_hu = pp.tile([S, D], F32, tag="small", name="ps_hu")
            nc.tensor.transpose(ps_hu[:], h_ownT[:], c32("i128"))
            h_own_u = wp.tile([S, D], F32, name="h_own_u")
            nc.vector.tensor_copy(h_own_u[:], ps_hu[:])
            ps_g = pp.tile([D, NG], F32, tag="small", name="ps_g")
            nc.tensor.matmul(ps_g[:], h_own_u[:], c32("meanMT_own"), start=True, stop=True)
            pg_s = wp.tile([D, NG], F32, name="pg_s")
            nc.vector.tensor_copy(pg_s[:], ps_g[:])
            ar_in = dp.tile([D, NG], F32, name="ar_in")
            ar_out = dp.tile([D, NG], F32, name="ar_out", addr_space="Shared")
            nc.sync.dma_start(ar_in[:], pg_s[:])
            nc.gpsimd.collective_compute(
                "AllReduce",
                ALU.add,
                replica_groups=[list(range(NC))],
                ins=[ar_in[:]],
                outs=[ar_out[:]],
            )

            # ---------- coord/attr heads (overlap the AllReduce) ----------
            ps_c = pp.tile([S, 2], F32, tag="small", name="ps_c")
            nc.tensor.matmul(ps_c[:], h_ownT[:], c32("cow"), start=True, stop=True)
            oc_s = wp.tile([S, 2], F32, name="oc_s")
            nc.vector.tensor_tensor(oc_s[:], ps_c[:], c32("cob"), op=ALU.add)
            nc.scalar.dma_start(o_coord, oc_s[:])

            ps_at = pp.tile([S, A], F32, tag="small", name="ps_at")
            nc.tensor.matmul(ps_at[:], h_ownT[:], c32("aow"), start=True, stop=True)
            oa_s = wp.tile([S, A], F32, name="oa_s")
            nc.vector.tensor_tensor(oa_s[:], ps_at[:], c32("aob"), op=ALU.add)
            nc.scalar.dma_start(o_attr, oa_s[:])

            # ---------- global MLP ----------
            hgT = wp.tile([D, NG], F32, name="hgT")
            nc.sync.dma_start(hgT[:], ar_out[:])
            ps_z1 = pp.tile([NG, D], F32, tag="small", name="ps_z1")
            nc.tensor.matmul(ps_z1[:], hgT[:], c32("gw1"), start=True, stop=True)
            z1 = wp.tile([NG, D], F32, name="z1")
            nc.vector.tensor_tensor(z1[:], ps_z1[:], c32("gb1"), op=ALU.add)
            z1r = wp.tile([NG, D], F32, name="z1r")
            nc.vector.tensor_scalar_max(z1r[:], z1[:], 0.0)
            ps_z1t = pp.tile([D, NG], F32, tag="small", name="ps_z1t")
            nc.tensor.transpose(ps_z1t[:], z1r[:], c32("i128")[:NG, :NG])
            z1t = wp.tile([D, NG], F32, name="z1t")
            nc.scalar.activation(z1t[:], ps_z1t[:], AF.Copy)
            ps_z2 = pp.tile([NG, G], F32, tag="small", name="ps_z2")
            nc.tensor.matmul(ps_z2[:], z1t[:], c32("gw2"), start=True, stop=True)
            og_s = wp.tile([NG, G], F32, name="og_s")
            nc.vector.tensor_tensor(og_s[:], ps_z2[:], c32("gb2"), op=ALU.add)
            nc.sync.dma_start(o_global, og_s[:])

    nc.compile()
    return nc


_PROGRAM = None


def _get_program():
    global _PROGRAM
    if _PROGRAM is None:
        _PROGRAM = _build_program()
    return _PROGRAM


def _build_in_maps(inputs):
    f32 = np.float32
    coord = np.asarray(inputs["theta_coord"], f32)        # [N, 2]
    attr = np.asarray(inputs["theta_attr"], f32)          # [N, A]
    t = np.asarray(inputs["t"], f32)                      # [N, 1]
    cond = np.asarray(inputs["condition"], f32)           # [N, D]
    batch = np.asarray(inputs["batch"]).astype(np.int64)  # [N]

    cw = np.asarray(inputs["coord_enc_W"], f32)
    aw = np.asarray(inputs["attr_enc_W"], f32)
    tw = np.asarray(inputs["time_W"], f32)
    enc_b = (
        np.asarray(inputs["coord_enc_b"], f32)
        + np.asarray(inputs["attr_enc_b"], f32)
        + np.asarray(inputs["time_b"], f32)
    )
    condTpb = cond.T + enc_b[:, None]                     # [D, N]

    wq = np.asarray(inputs["Wq"], f32)                    # [L, D, D]
    wk = np.asarray(inputs["Wk"], f32)
    wv = np.asarray(inputs["Wv"], f32)
    bq = np.asarray(inputs["bq"], f32)                    # [L, D]
    bk = np.asarray(inputs["bk"], f32)
    bv = np.asarray(inputs["bv"], f32)

    counts = np.bincount(batch, minlength=NG).astype(f32)
    onehot = (batch[None, :] == np.arange(NG)[:, None]).astype(f32)
    meanM = onehot / np.maximum(counts, 1.0)[:, None]     # [G, N]

    b32l, bbfl, s32l, sbfl = _LAYOUT

    # replicated bf16 blob
    bbf = _Blob(128, _BF)
    bbf.add("wq", np.concatenate([wq[l] for l in range(L)], 1).astype(_BF))
    bbf.add("wk", np.concatenate([wk[l] for l in range(L)], 1).astype(_BF))
    bbf.add("wv", np.concatenate([wv[l] for l in range(L)], 1).astype(_BF))
    bbf.add("condTpb", condTpb.astype(_BF))
    bbf.add("i128b", np.eye(D, dtype=f32).astype(_BF))
    blobbf = bbf.build()

    sbf = _Blob(16, _BF)
    sbf.add("coordT", coord.T.astype(_BF))
    sbf.add("attrT", attr.T.astype(_BF))
    sbf.add("tT", t.T.astype(_BF))
    sbf.add("cwb", cw.astype(_BF))
    sbf.add("awb", aw.astype(_BF))
    sbf.add("twb", tw.astype(_BF))
    smbf = sbf.build()

    in_maps = []
    for c in range(NC):
        sl = slice(c * S, (c + 1) * S)

        b32 = _Blob(128, f32)
        b32.add("condTpb_own", condTpb[:, sl])
        b32.add("i128", np.eye(D, dtype=f32))
        b32.add("cow", np.asarray(inputs["coord_out_W"], f32))
        b32.add("cob", np.broadcast_to(np.asarray(inputs["coord_out_b"], f32), (S, 2)))
        b32.add("aow", np.asarray(inputs["attr_out_W"], f32))
        b32.add("aob", np.broadcast_to(np.asarray(inputs["attr_out_b"], f32), (S, A)))
        b32.add("gw1", np.asarray(inputs["gW1"], f32))
        b32.add("gw2", np.asarray(inputs["gW2"], f32))
        b32.add("gb1", np.broadcast_to(np.asarray(inputs["gb1"], f32), (NG, D)))
        b32.add("gb2", np.broadcast_to(np.asarray(inputs["gb2"], f32), (NG, G)))
        b32.add("bq", bq.T)
        b32.add("bk", bk.T)
        b32.add("bv", bv.T)
        b32.add("meanMT_own", meanM[:, sl].T)
        blob32 = b32.build()

        s32 = _Blob(16, f32)
        s32.add("coordT_own", coord[sl].T)
        s32.add("attrT_own", attr[sl].T)
        s32.add("tT_own", t[sl].T)
        s32.add("cw", cw)
        s32.add("aw", aw)
        s32.add("tw", tw)
        s32.add("ones", np.ones((1, D), f32))
        sm32 = s32.build()

        in_maps.append(
            {"blob32": blob32, "blobbf": blobbf, "sm32": sm32, "smbf": smbf}
        )
    return in_maps


def kernel(**inputs):
    nc = _get_program()
    in_maps = _build_in_maps(inputs)
    res = bass_utils.run_bass_kernel_spmd(nc, in_maps, core_ids=list(range(NC)))
    coord_pred = np.concatenate(
        [res.results[c]["o_coord"] for c in range(NC)], axis=0
    )
    attr_pred = np.concatenate(
        [res.results[c]["o_attr"] for c in range(NC)], axis=0
    )
    global_pred = res.results[0]["o_global"]
    return (
        coord_pred.astype(np.float32),
        attr_pred.astype(np.float32),
        global_pred.astype(np.float32),
    )


# revision 11
# speedup vs baseline: 1.1227x; 1.0128x over previous
"""Trainium2 Bass kernel for nn_EquiStructureDecoder (8-core SPMD).

Key algebraic fact used: the network's outputs (coord_pred, attr_pred,
global_pred) depend only on the hidden stream h.  In each block,
h <- h + softmax(qk^T/sqrt(D)) @ v  uses only h; the coordinate stream x
(rel_x / edge_feat / coord_w / delta_x) never feeds back into h and is
not part of the returned outputs, so it is dead code and is not computed.
This is exact (dataflow equivalence of the h path), not an approximation.

Distribution (row-parallel over queries, per the sharding hint):
  - each of the 8 cores owns a 128-row slab of h (carried transposed,
    hT[d, i'], fp32 residual; bf16 operands for matmuls)
  - k/v are computed from the full (replicated) h each layer
  - after blocks 1 and 2 the updated transposed slabs are AllGather'd
    (bf16); the gather output is directly the stacked hT blocks
  - the global head needs only segment-sums of h, which are linear:
    each core computes its partial [D, G] and one 4KB AllReduce(add)
    replaces a full AllGather
  - softmax is computed without max-subtraction (scores for this model
    are O(0.1); exp is safe in fp32) and rows are normalized at the
    residual update via an outer-product broadcast of 1/rowsum
"""

import sys

for _p in ("/opt/trn_rl_repo",):
    if _p not in sys.path:
        sys.path.insert(0, _p)

import numpy as np
import ml_dtypes

import concourse.bass as bass
import concourse.bacc as bacc
import concourse.tile as tile
from concourse import mybir
from concourse import bass_utils

N = 1024
D = 128
NC = 8
S = N // NC        # 128 rows per core
L = 3
NG = 8
A = 16
G = 8
INV_SQRT_D = float(1.0 / np.sqrt(np.float32(D)))

F32 = mybir.dt.float32
BF16 = mybir.dt.bfloat16
AF = mybir.ActivationFunctionType
ALU = mybir.AluOpType

_BF = ml_dtypes.bfloat16


def _ts(i, size=128):
    return slice(i * size, (i + 1) * size)


class _Blob:
    """Column-packed constant blob: host array + SBUF slice bookkeeping."""

    def __init__(self, parts, dtype):
        self.parts = parts
        self.dtype = dtype
        self.cols = 0
        self.sections = {}
        self.arrays = []

    def add(self, name, arr):
        arr = np.asarray(arr)
        rows, cols = arr.shape
        assert rows <= self.parts
        self.sections[name] = (self.cols, cols, rows)
        self.cols += cols
        self.arrays.append(arr)
        return name

    def build(self):
        out = np.zeros((self.parts, self.cols), dtype=self.dtype)
        for (name, (off, cols, rows)), arr in zip(
            self.sections.items(), self.arrays
        ):
            out[:rows, off : off + cols] = arr
        return out


# blob section layouts (host + device must agree); filled in _build_in_maps
_B32 = None   # [128, *] f32, per-core
_BBF = None   # [128, *] bf16, replicated
_S32 = None   # [16, *] f32, per-core
_SBF = None   # [16, *] bf16, replicated


def _make_blob_layouts():
    """Define blob column layouts with dummy arrays (shapes only)."""
    b32 = _Blob(128, np.float32)
    b32.add("condTpb_own", np.zeros((D, S)))
    b32.add("i128", np.zeros((D, D)))
    b32.add("cow", np.zeros((D, 2)))
    b32.add("cob", np.zeros((S, 2)))
    b32.add("aow", np.zeros((D, A)))
    b32.add("aob", np.zeros((S, A)))
    b32.add("gw1", np.zeros((D, D)))
    b32.add("gw2", np.zeros((D, G)))
    b32.add("gb1", np.zeros((NG, D)))
    b32.add("gb2", np.zeros((NG, G)))
    b32.add("bq", np.zeros((D, L)))
    b32.add("bk", np.zeros((D, L)))
    b32.add("bv", np.zeros((D, L)))
    b32.add("meanMT_own", np.zeros((S, NG)))

    bbf = _Blob(128, _BF)
    bbf.add("wq", np.zeros((D, L * D)))
    bbf.add("wk", np.zeros((D, L * D)))
    bbf.add("wv", np.zeros((D, L * D)))
    bbf.add("condTpb", np.zeros((D, N)))
    bbf.add("i128b", np.zeros((D, D)))

    s32 = _Blob(16, np.float32)
    s32.add("coordT_own", np.zeros((2, S)))
    s32.add("attrT_own", np.zeros((A, S)))
    s32.add("tT_own", np.zeros((1, S)))
    s32.add("cw", np.zeros((2, D)))
    s32.add("aw", np.zeros((A, D)))
    s32.add("tw", np.zeros((1, D)))
    s32.add("ones", np.zeros((1, D)))

    sbf = _Blob(16, _BF)
    sbf.add("coordT", np.zeros((2, N)))
    sbf.add("attrT", np.zeros((A, N)))
    sbf.add("tT", np.zeros((1, N)))
    sbf.add("cwb", np.zeros((2, D)))
    sbf.add("awb", np.zeros((A, D)))
    sbf.add("twb", np.zeros((1, D)))
    return b32, bbf, s32, sbf


_LAYOUT = _make_blob_layouts()


def _build_program():
    b32l, bbfl, s32l, sbfl = _LAYOUT
    nc = bacc.Bacc(
        "TRN2",
        target_bir_lowering=False,
        debug=False,
        enable_asserts=False,
        num_devices=NC,
    )

    blob32 = nc.dram_tensor("blob32", [128, b32l.cols], F32, kind="ExternalInput").ap()
    blobbf = nc.dram_tensor("blobbf", [128, bbfl.cols], BF16, kind="ExternalInput").ap()
    sm32 = nc.dram_tensor("sm32", [16, s32l.cols], F32, kind="ExternalInput").ap()
    smbf = nc.dram_tensor("smbf", [16, sbfl.cols], BF16, kind="ExternalInput").ap()

    o_coord = nc.dram_tensor("o_coord", [S, 2], F32, kind="ExternalOutput").ap()
    o_attr = nc.dram_tensor("o_attr", [S, A], F32, kind="ExternalOutput").ap()
    o_global = nc.dram_tensor("o_global", [NG, G], F32, kind="ExternalOutput").ap()

    with tile.TileContext(nc) as tc:
        with (
            tc.tile_pool(name="const", bufs=1) as cp,
            tc.tile_pool(name="work", bufs=2) as wp,
            tc.tile_pool(name="psum", bufs=1, space="PSUM") as pp,
            tc.tile_pool(name="dram", bufs=1, space="DRAM") as dp,
        ):
            # ---------- constant blobs: 4 DMAs on 2 HWDGE queues ----------
            t32 = cp.tile([128, b32l.cols], F32, name="t32")
            tbf = cp.tile([128, bbfl.cols], BF16, name="tbf")
            u32 = cp.tile([16, s32l.cols], F32, name="u32")
            ubf = cp.tile([16, sbfl.cols], BF16, name="ubf")
            nc.scalar.dma_start(u32[:], sm32)
            nc.sync.dma_start(ubf[:], smbf)
            nc.scalar.dma_start(t32[:], blob32)
            nc.sync.dma_start(tbf[:], blobbf)

            def c32(name):
                off, cols, rows = b32l.sections[name]
                return t32[:rows, off : off + cols]

            def cbf(name):
                off, cols, rows = bbfl.sections[name]
                return tbf[:rows, off : off + cols]

            def c16(name):
                off, cols, rows = s32l.sections[name]
                return u32[:rows, off : off + cols]

            def c16b(name):
                off, cols, rows = sbfl.sections[name]
                return ubf[:rows, off : off + cols]

            def wsl(name, l):
                off, _, _ = bbfl.sections[name]
                return tbf[:, off + l * D : off + (l + 1) * D]

            def bsl(name, l):
                off, _, _ = b32l.sections[name]
                return t32[:, off + l : off + l + 1]

            # ---------- h0 ----------
            # full h0 (bf16) straight into the hT layout used by k/v
            hT_all = cp.tile([D, N], BF16, name="hT_all0")
            for c in range(2):
                psf = pp.tile([D, 512], F32, tag="big", bufs=3, name="psf")
                sl = slice(c * 512, (c + 1) * 512)
                nc.tensor.matmul(psf[:], c16b("cwb"), c16b("coordT")[:, sl], start=True, stop=False)
                nc.tensor.matmul(psf[:], c16b("awb"), c16b("attrT")[:, sl], start=False, stop=False)
                nc.tensor.matmul(psf[:], c16b("twb"), c16b("tT")[:, sl], start=False, stop=True)
                nc.vector.tensor_tensor(hT_all[:, sl], psf[:], cbf("condTpb")[:, sl], op=ALU.add)

            # own slab fp32 (exact residual carry)
            h_ownT = cp.tile([D, S], F32, name="h_ownT")
            ps0 = pp.tile([D, S], F32, tag="small", bufs=2, name="ps0")
            nc.tensor.matmul(ps0[:], c16("cw"), c16("coordT_own"), start=True, stop=False)
            nc.tensor.matmul(ps0[:], c16("aw"), c16("attrT_own"), start=False, stop=False)
            nc.tensor.matmul(ps0[:], c16("tw"), c16("tT_own"), start=False, stop=True)
            nc.vector.tensor_tensor(h_ownT[:], ps0[:], c32("condTpb_own"), op=ALU.add)
            h_ownT_b = cp.tile([D, S], BF16, name="h_ownT_b0")
            nc.scalar.activation(h_ownT_b[:], h_ownT[:], AF.Copy)

            # ---------- attention blocks ----------
            for l in range(L):
                last = l == L - 1

                if l > 0:
                    # rebuild hT_all from the AllGather output (stacked
                    # transposed blocks); split across both HWDGE queues
                    hT_all = wp.tile([D, N], BF16, name="hT_all")
                    for b in range(NC):
                        eng = nc.sync if b % 2 == 0 else nc.scalar
                        eng.dma_start(hT_all[:, _ts(b)], ag_out[_ts(b), :])

                # q (own): qT[d,i'] = Wq^T @ h_ownT (+bq); ready during AG
                ps_q = pp.tile([D, S], F32, tag="small", bufs=2, name="ps_q")
                nc.tensor.matmul(ps_q[:], wsl("wq", l), h_ownT_b[:], start=True, stop=True)
                qT_b = wp.tile([D, S], BF16, name="qT_b")
                nc.scalar.activation(qT_b[:], ps_q[:], AF.Identity, bias=bsl("bq", l))

                # pipelined by j-halves: kT -> S -> exp -> transpose -> ET
                # -> agg; half 0's tail overlaps half 1's head
                ps_k = pp.tile([D, N], F32, tag="big", bufs=3, name="ps_k")
                kT_b = wp.tile([D, N], BF16, name="kT_b")
                ps_s = pp.tile([S, N], F32, tag="big", bufs=3, name="ps_s")
                ps_v = pp.tile([D, N], F32, tag="big", bufs=3, name="ps_v")
                v_b = wp.tile([D, N], BF16, name="v_b")
                e_b = wp.tile([S, N], BF16, name="e_b")
                rs2 = wp.tile([S, 2], F32, name="rs2")
                ps_et = pp.tile([S, N], BF16, tag="big", bufs=3, name="ps_et")
                et_b = wp.tile([S, N], BF16, name="et_b")
                ps_a = pp.tile([D, S], F32, tag="small", bufs=2, name="ps_a")
                rowsum = wp.tile([S, 1], F32, name="rowsum")
                recip = wp.tile([S, 1], F32, name="recip")

                for c in range(2):
                    sl = slice(c * 512, (c + 1) * 512)
                    nc.tensor.matmul(ps_k[:, sl], wsl("wk", l), hT_all[:, sl], start=True, stop=True)
                    nc.vector.tensor_scalar_add(kT_b[:, sl], ps_k[:, sl], bsl("bk", l))
                    nc.tensor.matmul(ps_s[:, sl], qT_b[:], kT_b[:, sl], start=True, stop=True)
                    # E = exp(S/sqrt(D)) unnormalized (bf16) + fp32 half-rowsum
                    nc.scalar.activation(
                        e_b[:, sl], ps_s[:, sl], AF.Exp, scale=INV_SQRT_D,
                        accum_out=rs2[:, c : c + 1],
                    )
                    # v[j,d] for this half (bv folded into the update)
                    for b in range(4 * c, 4 * (c + 1)):
                        nc.tensor.matmul(ps_v[:, _ts(b)], hT_all[:, _ts(b)], wsl("wv", l), start=True, stop=True)
                    nc.scalar.activation(v_b[:, sl], ps_v[:, sl], AF.Copy)
                    # transpose unnormalized E blocks of this half
                    for b in range(4 * c, 4 * (c + 1)):
                        nc.tensor.transpose(ps_et[:, _ts(b)], e_b[:, _ts(b)], cbf("i128b"))
                    nc.vector.tensor_copy(et_b[:, sl], ps_et[:, sl])
                    # aggT[d,i'] += sum_b v_b^T @ ET_b  == (E @ v)^T
                    # (two closed accumulation groups onto the same bank;
                    # the second opens with start=False to keep accumulating)
                    for b in range(4 * c, 4 * (c + 1)):
                        nc.tensor.matmul(
                            ps_a[:], v_b[:, _ts(b)], et_b[:, _ts(b)],
                            start=(b == 0), stop=(b % 4 == 3),
                            skip_group_check=True,
                        )

                # recip broadcast tile rb[d, i'] = 1/rowsum[i']  (off-chain)
                nc.vector.tensor_reduce(rowsum[:], rs2[:], axis=mybir.AxisListType.X, op=ALU.add)
                nc.vector.reciprocal(recip[:], rowsum[:])
                ps_rt = pp.tile([1, S], F32, tag="small", bufs=2, name="ps_rt")
                nc.tensor.transpose(ps_rt[:], recip[:], c32("i128"))
                rt_s = wp.tile([1, S], F32, name="rt_s")
                nc.vector.tensor_copy(rt_s[:], ps_rt[:])
                ps_rb = pp.tile([D, S], F32, tag="small", bufs=2, name="ps_rb")
                nc.tensor.matmul(ps_rb[:], c16("ones"), rt_s[:], start=True, stop=True)
                rb_s = wp.tile([D, S], F32, name="rb_s")
                nc.vector.tensor_copy(rb_s[:], ps_rb[:])

                # h <- h + aggT*rb + bv; bf16 copy first (feeds q + AllGather)
                t1 = wp.tile([D, S], F32, name="t1")
                nc.vector.tensor_tensor(t1[:], ps_a[:], rb_s[:], op=ALU.mult)
                h_ownT_b = wp.tile([D, S], BF16, name="h_ownT_b")
                nc.vector.scalar_tensor_tensor(
                    h_ownT_b[:], t1[:], bsl("bv", l), h_ownT[:], op0=ALU.add, op1=ALU.add
                )
                if not last:
                    ag_in = dp.tile([D, S], BF16, name=f"ag_in{l}")
                    ag_out = dp.tile([N, S], BF16, name=f"ag_out{l}", addr_space="Shared")
                    nc.gpsimd.dma_start(ag_in[:], h_ownT_b[:])
                    nc.gpsimd.collective_compute(
                        "AllGather",
                        ALU.bypass,
                        replica_groups=[list(range(NC))],
                        ins=[ag_in[:]],
                        outs=[ag_out[:]],
                    )
                h_new = cp.tile([D, S], F32, name=f"h_new{l}")
                nc.vector.scalar_tensor_tensor(
                    h_new[:], t1[:], bsl("bv", l), h_ownT[:], op0=ALU.add, op1=ALU.add
                )
                h_ownT = h_new

            # ---------- global head partial + AllReduce (fires first) ----------
            ps_hu = pp.tile([S, D], F32, tag="small", bufs=2, name="ps_hu")
            nc.tensor.transpose(ps_hu[:], h_ownT[:], c32("i128"))
            h_own_u = wp.tile([S, D], F32, name="h_own_u")
            nc.vector.tensor_copy(h_own_u[:], ps_hu[:])
            ps_g = pp.tile([D, NG], F32, tag="small", bufs=2, name="ps_g")
            nc.tensor.matmul(ps_g[:], h_own_u[:], c32("meanMT_own"), start=True, stop=True)
            pg_s = wp.tile([D, NG], F32, name="pg_s")
            nc.vector.tensor_copy(pg_s[:], ps_g[:])
            ar_in = dp.tile([D, NG], F32, name="ar_in")
            ar_out = dp.tile([D, NG], F32, name="ar_out", addr_space="Shared")
            nc.sync.dma_start(ar_in[:], pg_s[:])
            nc.gpsimd.collective_compute(
                "AllReduce",
                ALU.add,
                replica_groups=[list(range(NC))],
                ins=[ar_in[:]],
                outs=[ar_out[:]],
            )

            # ---------- coord/attr heads (overlap the AllReduce) ----------
            ps_c = pp.tile([S, 2], F32, tag="small", bufs=2, name="ps_c")
            nc.tensor.matmul(ps_c[:], h_ownT[:], c32("cow"), start=True, stop=True)
            oc_s = wp.tile([S, 2], F32, name="oc_s")
            nc.vector.tensor_tensor(oc_s[:], ps_c[:], c32("cob"), op=ALU.add)
            nc.scalar.dma_start(o_coord, oc_s[:])

            ps_at = pp.tile([S, A], F32, tag="small", bufs=2, name="ps_at")
            nc.tensor.matmul(ps_at[:], h_ownT[:], c32("aow"), start=True, stop=True)
            oa_s = wp.tile([S, A], F32, name="oa_s")
            nc.vector.tensor_tensor(oa_s[:], ps_at[:], c32("aob"), op=ALU.add)
            nc.scalar.dma_start(o_attr, oa_s[:])

            # ---------- global MLP ----------
            hgT = wp.tile([D, NG], F32, name="hgT")
            nc.sync.dma_start(hgT[:], ar_out[:])
            ps_z1 = pp.tile([NG, D], F32, tag="small", bufs=2, name="ps_z1")
            nc.tensor.matmul(ps_z1[:], hgT[:], c32("gw1"), start=True, stop=True)
            z1 = wp.tile([NG, D], F32, name="z1")
            nc.vector.tensor_tensor(z1[:], ps_z1[:], c32("gb1"), op=ALU.add)
            z1r = wp.tile([NG, D], F32, name="z1r")
            nc.vector.tensor_scalar_max(z1r[:], z1[:], 0.0)
            ps_z1t = pp.tile([D, NG], F32, tag="small", bufs=2, name="ps_z1t")
            nc.tensor.transpose(ps_z1t[:], z1r[:], c32("i128")[:NG, :NG])
            z1t = wp.tile([D, NG], F32, name="z1t")
            nc.scalar.activation(z1t[:], ps_z1t[:], AF.Copy)
            ps_z2 = pp.tile([NG, G], F32, tag="small", bufs=2, name="ps_z2")
            nc.tensor.matmul(ps_z2[:], z1t[:], c32("gw2"), start=True, stop=True)
            og_s = wp.tile([NG, G], F32, name="og_s")
            nc.vector.tensor_tensor(og_s[:], ps_z2[:], c32("gb2"), op=ALU.add)
            nc.sync.dma_start(o_global, og_s[:])

    nc.compile()
    return nc


_PROGRAM = None


def _get_program():
    global _PROGRAM
    if _PROGRAM is None:
        _PROGRAM = _build_program()
    return _PROGRAM


def _build_in_maps(inputs):
    f32 = np.float32
    coord = np.asarray(inputs["theta_coord"], f32)        # [N, 2]
    attr = np.asarray(inputs["theta_attr"], f32)          # [N, A]
    t = np.asarray(inputs["t"], f32)                      # [N, 1]
    cond = np.asarray(inputs["condition"], f32)           # [N, D]
    batch = np.asarray(inputs["batch"]).astype(np.int64)  # [N]

    cw = np.asarray(inputs["coord_enc_W"], f32)
    aw = np.asarray(inputs["attr_enc_W"], f32)
    tw = np.asarray(inputs["time_W"], f32)
    enc_b = (
        np.asarray(inputs["coord_enc_b"], f32)
        + np.asarray(inputs["attr_enc_b"], f32)
        + np.asarray(inputs["time_b"], f32)
    )
    condTpb = cond.T + enc_b[:, None]                     # [D, N]

    wq = np.asarray(inputs["Wq"], f32)                    # [L, D, D]
    wk = np.asarray(inputs["Wk"], f32)
    wv = np.asarray(inputs["Wv"], f32)
    bq = np.asarray(inputs["bq"], f32)                    # [L, D]
    bk = np.asarray(inputs["bk"], f32)
    bv = np.asarray(inputs["bv"], f32)

    counts = np.bincount(batch, minlength=NG).astype(f32)
    onehot = (batch[None, :] == np.arange(NG)[:, None]).astype(f32)
    meanM = onehot / np.maximum(counts, 1.0)[:, None]     # [G, N]

    b32l, bbfl, s32l, sbfl = _LAYOUT

    # replicated bf16 blob
    bbf = _Blob(128, _BF)
    bbf.add("wq", np.concatenate([wq[l] for l in range(L)], 1).astype(_BF))
    bbf.add("wk", np.concatenate([wk[l] for l in range(L)], 1).astype(_BF))
    bbf.add("wv", np.concatenate([wv[l] for l in range(L)], 1).astype(_BF))
    bbf.add("condTpb", condTpb.astype(_BF))
    bbf.add("i128b", np.eye(D, dtype=f32).astype(_BF))
    blobbf = bbf.build()

    sbf = _Blob(16, _BF)
    sbf.add("coordT", coord.T.astype(_BF))
    sbf.add("attrT", attr.T.astype(_BF))
    sbf.add("tT", t.T.astype(_BF))
    sbf.add("cwb", cw.astype(_BF))
    sbf.add("awb", aw.astype(_BF))
    sbf.add("twb", tw.astype(_BF))
    smbf = sbf.build()

    in_maps = []
    for c in range(NC):
        sl = slice(c * S, (c + 1) * S)

        b32 = _Blob(128, f32)
        b32.add("condTpb_own", condTpb[:, sl])
        b32.add("i128", np.eye(D, dtype=f32))
        b32.add("cow", np.asarray(inputs["coord_out_W"], f32))
        b32.add("cob", np.broadcast_to(np.asarray(inputs["coord_out_b"], f32), (S, 2)))
        b32.add("aow", np.asarray(inputs["attr_out_W"], f32))
        b32.add("aob", np.broadcast_to(np.asarray(inputs["attr_out_b"], f32), (S, A)))
        b32.add("gw1", np.asarray(inputs["gW1"], f32))
        b32.add("gw2", np.asarray(inputs["gW2"], f32))
        b32.add("gb1", np.broadcast_to(np.asarray(inputs["gb1"], f32), (NG, D)))
        b32.add("gb2", np.broadcast_to(np.asarray(inputs["gb2"], f32), (NG, G)))
        b32.add("bq", bq.T)
        b32.add("bk", bk.T)
        b32.add("bv", bv.T)
        b32.add("meanMT_own", meanM[:, sl].T)
        blob32 = b32.build()

        s32 = _Blob(16, f32)
        s32.add("coordT_own", coord[sl].T)
        s32.add("attrT_own", attr[sl].T)
        s32.add("tT_own", t[sl].T)
        s32.add("cw", cw)
        s32.add("aw", aw)
        s32.add("tw", tw)
        s32.add("ones", np.ones((1, D), f32))
        sm32 = s32.build()

        in_maps.append(
            {"blob32": blob32, "blobbf": blobbf, "sm32": sm32, "smbf": smbf}
        )
    return in_maps


def kernel(**inputs):
    nc = _get_program()
    in_maps = _build_in_maps(inputs)
    res = bass_utils.run_bass_kernel_spmd(nc, in_maps, core_ids=list(range(NC)))
    coord_pred = np.concatenate(
        [res.results[c]["o_coord"] for c in range(NC)], axis=0
    )
    attr_pred = np.concatenate(
        [res.results[c]["o_attr"] for c in range(NC)], axis=0
    )
    global_pred = res.results[0]["o_global"]
    return (
        coord_pred.astype(np.float32),
        attr_pred.astype(np.float32),
        global_pred.astype(np.float32),
    )
